# revision 34
# baseline (speedup 1.0000x reference)
"""Trainium2 Bass kernel for fused attention prefill (nn_Attn_50740743635107).

Reference computation (fp32):
  qkv = x @ W_qkv.T ; split q,k,v ; interleaved RoPE on q,k ;
  scores = q k^T / sqrt(dh) with causal+valid_k mask ; softmax ;
  ctx = attn @ v ; out = ctx @ W_out.T
Shapes: B=4, S=1024, D=2048, H=16, DH=128.

Sharding: 8 cores = 4 batches x 2 head-groups (8 heads each).
Each core computes a partial out^T [D, S] for its (batch, head-group);
the host sums the two head-group partials per batch and transposes.

v3 design notes (on top of the bf16 v2 pipeline):
- q/k projection runs in fp8e4m3 with MatmulPerfMode.DoubleRow: the
  contraction pairs two 128-d chunks per matmul ([128,2,N] moving APs),
  so the PE contracts 256 rows per 1-cycle column stream - 2x the bf16
  rate.  Measured on HW: a DoubleRow matmul costs the same ~1 cyc/col
  as bf16 while doing double the work; plain fp8 and 64-partition
  DoubleRow give no speedup, so scores stay bf16.
- fp8 error budget: e4m3 rounding is ~2.5% rms per operand.  q/k noise
  only perturbs softmax scores (common-mode cancels in the softmax
  ratio), measured end-to-end ~7e-3 rel err.  The v path (v-proj, ctx,
  out-proj) goes STRAIGHT to the output, where a single fp8 operand
  already measures ~2.5e-2 > 2e-2 tol, so it stays bf16.
- W_q/W_k are quantized at x256 (std 0.0156 -> 4.0; e4m3 subnormal
  floor is 2^-6) and descaled in the PSUM->SBUF copy scale; DH^-0.5
  rides the q copy scale.
- Everything else per v2: bf16 PE/DVE paths, f32 PSUM, host-side
  transposed layouts, k-projection truncated at 928 tokens, one exp
  per score tile with bias-mask, causal mask via tri-multiply on the
  diagonal block after exp, softmax denominator via ones-stationary
  matmul, reciprocal_approx_fast, PE p-state warm-up matmuls, sh-major
  attention with the sq<512 output projection interleaved in.

Scheduling (measured on HW, exec 324.5us -> 240.4us):
- The first ~20us is chip-aggregate DMA-bound (8 cores pulling inputs
  at once).  Only x8 + wqk8 issue early, spread over three queues; the
  v-proj inputs (xT bf16, wv) queue behind the late wqk8 tiles on
  gpsimd; wom rides sync alone.  40 warm-up matmuls bridge the landing
  window (also ramping the PE p-state).
- The warm-up SBUF tile's pool stays open through phase A: if it
  closed, the x8/wqk8 tiles would reuse its space and their DMAs would
  serialize behind the warm-up matmuls (cost: 14us, measured).
- qk-projection runs 2 m-tiles per block (psa bufs=6) so each landed
  x8 chunk feeds 4 matmuls.
- The last v-proj PSUM copies ride DVE so ACT is free for the first
  attention exps; drain-tail stores avoid gpsimd (slow queue DRAIN)
  and the last two output tiles split 512->2x256 cols to overlap
  copy/store with matmuls.
"""

import numpy as np
import ml_dtypes

import concourse.bass as bass
from concourse import bacc
import concourse.mybir as mybir
import concourse.tile as tile
from concourse.bass_utils import run_bass_kernel_spmd

B, S, D, H = 4, 1024, 2048, 16
DH = 128           # head dim
HPC = 8            # heads per core
DC = HPC * DH      # 1024: d-range per core
P = 128
KTOK = 928         # k tokens computed (>= max seq_len, multiple of 16)
THETA = 10000.0
NEG = -60.0        # additive mask for invalid-k positions
WQK_SCALE = 256.0  # fp8 quantization scale for W_q/W_k rows
F32 = mybir.dt.float32
BF16 = mybir.dt.bfloat16
F8 = mybir.dt.float8e4
DR = mybir.MatmulPerfMode.DoubleRow
MULT = mybir.AluOpType.mult
ADD = mybir.AluOpType.add
SUB = mybir.AluOpType.subtract
EXP = mybir.ActivationFunctionType.Exp
COPY = mybir.ActivationFunctionType.Copy

# score tiles per head: (sk_tile, sq_half) pairs that are (partially) allowed.
# sq_half h covers sq in [512h, 512h+512); sk tile t covers sk in [128t, ...).
# diagonal (need causal mask): sh=0: t=0..3 ; sh=1: t=4..7.
# full (no mask): sh=1: t=0..3.
DIAG = {(t, 0) for t in range(4)} | {(t, 1) for t in range(4, 8)}
ALLOWED = {0: [0, 1, 2, 3], 1: [0, 1, 2, 3, 4, 5, 6, 7]}  # sq_half -> sk tiles


def build_nc(ktok=KTOK):
    nc = bacc.Bacc()
    xT_d = nc.dram_tensor("xT", [D, S], BF16, kind="ExternalInput")
    # x8[p, j, i, s] = xT[256j + 128i + p, s] quantized e4m3: pair-interleaved
    # chunks for DoubleRow (contraction = 128 partitions x 2)
    x8_d = nc.dram_tensor("x8", [P, 8, 2, S], F8, kind="ExternalInput")
    # wqk8[p, mt, j, i, m] = WqkT[256j+128i+p, 128mt+m] * WQK_SCALE (e4m3)
    wqk8_d = nc.dram_tensor("wqk8", [P, 16, 8, 2, P], F8, kind="ExternalInput")
    # wvm[p, mtv, kt, m] = WvT[128kt+p, 128mtv+m] (bf16)
    wvm_d = nc.dram_tensor("wvm", [P, 8, 16, P], BF16, kind="ExternalInput")
    wom_d = nc.dram_tensor("wom", [P, 8, D], BF16, kind="ExternalInput")
    cs_d = nc.dram_tensor("cs", [P, 2, S], BF16, kind="ExternalInput")
    tri_d = nc.dram_tensor("tri", [P, P], BF16, kind="ExternalInput")
    bias_d = nc.dram_tensor("bias", [P, 8], F32, kind="ExternalInput")
    ones_d = nc.dram_tensor("ones", [P, P], BF16, kind="ExternalInput")
    outT_d = nc.dram_tensor("outT", [D, S], BF16, kind="ExternalOutput")

    with tile.TileContext(nc) as tc:
        with (
            tc.tile_pool(name="qkt", bufs=1) as qktp,      # [128,16,1024] bf16 32K/p
            tc.tile_pool(name="vsb", bufs=1) as vsbp,      # [128,8,1024] bf16 16K/p
            tc.tile_pool(name="cstb", bufs=1) as cstbp,    # tri/bias/ones consts
            # warm-up SBUF tile lives in the OUTER scope: if its pool closed,
            # phase A's x8/wqk8 tiles would reuse its space and their DMAs
            # would stall behind all the warm-up matmuls (measured 14us of
            # PE starvation from exactly that).
            tc.tile_pool(name="wsb", bufs=1) as wsbp,
        ):
            qkT = qktp.tile([P, 16, S], BF16, tag="qkt")
            vsb = vsbp.tile([P, 8, DC], BF16, tag="vsb")
            # consts on the gpsimd queue (cheap issue) so the scalar queue
            # starts streaming x8/wqk8 immediately
            tri_t = cstbp.tile([P, P], BF16, tag="tri")
            nc.gpsimd.dma_start(tri_t[:], tri_d[:])
            bias_t = cstbp.tile([P, 8], F32, tag="bias")
            nc.gpsimd.dma_start(bias_t[:], bias_d[:])
            ones_t = cstbp.tile([P, P], BF16, tag="ones")
            nc.gpsimd.dma_start(ones_t[:], ones_d[:])

            # ---- PE p-state warm-up: dummy matmuls on a memset tile (no DMA
            # dependency) while the input DMAs land. Output is never read.
            with (
                tc.tile_pool(name="warm", bufs=2, space=bass.MemorySpace.PSUM) as wps,
            ):
                warm_sb = wsbp.tile([P, 512], BF16, tag="wsb")
                nc.vector.memset(warm_sb[:], 0.5)
                for w in range(40):
                    wp = wps.tile([P, 512], F32, tag="warm")
                    nc.tensor.matmul(
                        wp[:], warm_sb[:, 0:P], warm_sb[:], start=True, stop=True
                    )

            # ================= phase A: QKV projection + RoPE =================
            with (
                tc.tile_pool(name="x8t", bufs=8) as x8p,     # 8x[128,2,1024] f8 2K/p
                tc.tile_pool(name="xt", bufs=16) as xtp,     # 16x[128,1024] bf16 32K/p
                tc.tile_pool(name="wqk", bufs=8) as wqkp,    # [128,8,2,128] f8 2K/p
                tc.tile_pool(name="wv", bufs=2) as wvp,      # [128,4,16,128] bf16 16K/p
                tc.tile_pool(name="cst", bufs=1) as cstp,    # cos/sin 4K/p
                tc.tile_pool(name="rope", bufs=2) as ropep,  # [128,1024] bf16 2K/p
                tc.tile_pool(name="psa", bufs=6, space=bass.MemorySpace.PSUM) as psap,
                tc.tile_pool(name="psv", bufs=2, space=bass.MemorySpace.PSUM) as psvp,
            ):
                # x8 as 8 SEPARATE tiles for per-chunk DMA dependencies: the
                # first DoubleRow matmul gates only on x8_0 + wqk8_0.
                x8s = [
                    x8p.tile([P, 2, S], F8, tag="x8t", name=f"x8_{j}")
                    for j in range(8)
                ]
                # bf16 x tiles feed the v projection (stationary slices)
                xts = [
                    xtp.tile([P, S], BF16, tag="xt", name=f"xt{kt}")
                    for kt in range(16)
                ]
                # zero the uncomputed k tail (tokens ktok..S of every k head)
                if ktok < S:
                    nc.vector.memset(qkT[:, 8:16, ktok:S], 0.0)
                # DMA plan: the first ~45us of HBM bandwidth is chip-aggregate
                # bound (8 cores pulling at once), and the qk-projection is
                # rate-limited by x8/wqk8 landing.  So ONLY x8 + wqk8 issue
                # early (scalar + gpsimd-after-consts); the v-proj inputs
                # (xts, wv) queue up on gpsimd BEHIND the late wqk8 tiles,
                # and wom rides sync alone (needed last).
                cs_t = cstp.tile([P, 2, S], BF16, tag="cs")
                nc.gpsimd.dma_start(cs_t[:], cs_d[:])
                wqks = []
                with tc.high_priority():
                    wqk_p = wqkp.tile([P, 8, 2, P], F8, tag="wqk", name="wqk_pre0")
                    nc.scalar.dma_start(wqk_p[:], wqk8_d[:, 0])
                    wqks.append(wqk_p)
                    # x8 spreads across three queues so parallel DGE streams
                    # land the qk-projection inputs faster
                    for j in range(8):
                        eng = (nc.scalar, nc.sync, nc.scalar, nc.gpsimd)[j % 4]
                        eng.dma_start(x8s[j][:], x8_d[:, j])
                    for pre in range(1, 6):
                        wqk_p = wqkp.tile(
                            [P, 8, 2, P], F8, tag="wqk", name=f"wqk_pre{pre}"
                        )
                        eng = nc.gpsimd if pre >= 3 else nc.scalar
                        eng.dma_start(wqk_p[:], wqk8_d[:, pre])
                        wqks.append(wqk_p)

                # ---- q/k projection in fp8 DoubleRow
                # (m-tile mt: 0..7 = q heads, 8..15 = k heads)
                # qkT[m, s] = sum_d wqkT[d, m] * xT[d, s]
                # Two m-tiles per block share each landed x8 chunk: 4 matmuls
                # (1.7us of PE work) per x8_j instead of 2, hiding the
                # early-window DMA landing rate.
                for blk in range(8):
                    mts = (2 * blk, 2 * blk + 1)
                    wqkts = []
                    for mt in mts:
                        if mt < 6:
                            wqkts.append(wqks[mt])
                        else:
                            w = wqkp.tile([P, 8, 2, P], F8, tag="wqk")
                            nc.gpsimd.dma_start(w[:], wqk8_d[:, mt])
                            wqkts.append(w)
                    is_k = mts[0] >= 8
                    n1 = (ktok - 512) if is_k else 512  # second-half cols
                    pss = [
                        (
                            psap.tile([P, 512], F32, tag="psa", name=f"qk{mt}_0"),
                            psap.tile([P, 512], F32, tag="psa", name=f"qk{mt}_1"),
                        )
                        for mt in mts
                    ]
                    for j in range(8):
                        for w, (ps0, ps1) in zip(wqkts, pss):
                            nc.tensor.matmul(
                                ps0[:], w[:, j], x8s[j][:, :, 0:512],
                                start=(j == 0), stop=(j == 7), perf_mode=DR,
                            )
                            nc.tensor.matmul(
                                ps1[:, 0:n1], w[:, j], x8s[j][:, :, 512 : 512 + n1],
                                start=(j == 0), stop=(j == 7), perf_mode=DR,
                            )
                    # descale fp8 W scale; q additionally gets 1/sqrt(dh)
                    csc = (1.0 / WQK_SCALE) * (1.0 if is_k else DH**-0.5)
                    for mt, (ps0, ps1) in zip(mts, pss):
                        nc.scalar.activation(
                            qkT[:, mt, 0:512], ps0[:], COPY, scale=csc
                        )
                        nc.scalar.activation(
                            qkT[:, mt, 512 : 512 + n1], ps1[:, 0:n1], COPY, scale=csc
                        )
                        # ---- RoPE in place on qkT[:, mt, :] once both halves
                        # done.  rows 0..63 = even dh (xe), 64..127 = odd (xo):
                        #   new_e = xe*cos - xo*sin ; new_o = xe*sin + xo*cos
                        tmp = ropep.tile([P, S], BF16, tag="rope")
                        col = qkT[:, mt, :]
                        nc.vector.tensor_tensor(
                            tmp[0:64, :], col[64:128, :], cs_t[64:128, 1, :], op=MULT
                        )
                        nc.vector.tensor_tensor(
                            tmp[64:128, :], col[0:64, :], cs_t[0:64, 1, :], op=MULT
                        )
                        nc.vector.tensor_tensor(col[:], col[:], cs_t[:, 0, :], op=MULT)
                        nc.vector.tensor_tensor(
                            col[0:64, :], col[0:64, :], tmp[0:64, :], op=SUB
                        )
                        nc.vector.tensor_tensor(
                            col[64:128, :], col[64:128, :], tmp[64:128, :], op=ADD
                        )

                # v-proj inputs issue AFTER the qk weights in the gpsimd
                # queue so they don't steal early HBM bandwidth; they land
                # by the time the PE finishes the 16 qk m-tiles.
                for kt in range(16):
                    nc.gpsimd.dma_start(xts[kt][:], xT_d[P * kt : P * (kt + 1), :])

                # ---- v projection: v[s, vd] = sum_d xT[d, s] * wvT[d, vd]
                # 512-wide moving chunks (full PSUM bank per matmul)
                for nh in range(2):
                    wv = wvp.tile([P, 4, 16, P], BF16, tag="wv")
                    nc.gpsimd.dma_start(wv[:], wvm_d[:, 4 * nh : 4 * (nh + 1)])
                    for st in range(8):
                        psv = psvp.tile([P, 512], F32, tag="psv")
                        for kt in range(16):
                            nc.tensor.matmul(
                                psv[:],
                                xts[kt][:, P * st : P * (st + 1)],
                                wv[:, :, kt, :],
                                start=(kt == 0),
                                stop=(kt == 15),
                            )
                        # the last few copies ride DVE so ACT is free for the
                        # first attention exps the moment their scores land
                        if nh == 1 and st >= 4:
                            nc.vector.tensor_copy(
                                vsb[:, st, 512 * nh : 512 * (nh + 1)], psv[:]
                            )
                        else:
                            nc.scalar.activation(
                                vsb[:, st, 512 * nh : 512 * (nh + 1)], psv[:], COPY
                            )

            # ============ phase B: attention + output projection ============
            with (
                tc.tile_pool(name="ctx", bufs=1) as ctxp,    # [128,8,1024] bf16 16K/p
                tc.tile_pool(name="ex", bufs=4) as exps,     # [128,512] bf16 1K/p
                tc.tile_pool(name="rc", bufs=2) as rcp,      # [128,512] f32 2K/p
                tc.tile_pool(name="wo", bufs=1) as wop,      # [128,8,2048] bf16 32K/p
                tc.tile_pool(name="ot", bufs=3) as otp,      # [128,512] bf16 1K/p
                tc.tile_pool(name="ps", bufs=4, space=bass.MemorySpace.PSUM) as psp,
                tc.tile_pool(name="psc", bufs=1, space=bass.MemorySpace.PSUM) as pscp,
                tc.tile_pool(name="psd", bufs=1, space=bass.MemorySpace.PSUM) as psdp,
                tc.tile_pool(name="pso", bufs=2, space=bass.MemorySpace.PSUM) as psop,
            ):
                ctxT = ctxp.tile([P, 8, S], BF16, tag="ctx")
                # W_out^T resident for the output projection
                wo_t = wop.tile([P, 8, D], BF16, tag="wo")
                nc.sync.dma_start(wo_t[:], wom_d[:])

                # ---- attention, software-pipelined and sh-major: scores
                # issue LOOKAHEAD items ahead of their exp/ctx/den; the sh=0
                # half of the output projection is interleaved into the sh=1
                # attention stream to keep the PE busy under the ACT chain.
                work = []  # (h, sh, t, i, ntiles)
                for sh in range(2):
                    for h in range(8):
                        tiles = ALLOWED[sh]
                        for i, t in enumerate(tiles):
                            work.append((h, sh, t, i, len(tiles)))

                LOOKAHEAD = 3
                scs = {}
                groups = {}  # (h, sh) -> (ctx_ps, den_ps)

                def issue_score(j):
                    h, sh, t, i, _n = work[j]
                    c0 = P * t - 512 * sh if (t, sh) in DIAG else 0
                    # the first two scores borrow the out-proj PSUM banks
                    # (idle until item ~56) so the pipeline fill runs 5 deep
                    # instead of 3 without overflowing psp afterwards
                    pool = psop if j < 2 else psp
                    sc = pool.tile([P, 512], F32, tag="pso" if j < 2 else "ps")
                    nc.tensor.matmul(
                        sc[:, c0:512],
                        qkT[:, 8 + h, P * t : P * (t + 1)],
                        qkT[:, h, 512 * sh + c0 : 512 * (sh + 1)],
                        start=True,
                        stop=True,
                    )
                    scs[j] = sc

                def out_proj(me, sh, nsplit=1):
                    # outT[e, sq] = sum_d woT[d, e] * ctxT[d, sq]
                    # nsplit=2 chops the 512 columns into two accumulation
                    # groups so the copy+store of the first half overlaps the
                    # second half's matmuls (shortens the drain tail).
                    w = 512 // nsplit
                    for hf in range(nsplit):
                        q0 = 512 * sh + w * hf
                        ps = psop.tile([P, w], F32, tag="pso")
                        for kd in range(8):
                            nc.tensor.matmul(
                                ps[:],
                                wo_t[:, kd, P * me : P * (me + 1)],
                                ctxT[:, kd, q0 : q0 + w],
                                start=(kd == 0),
                                stop=(kd == 7),
                            )
                        ot = otp.tile([P, w], BF16, tag="ot")
                        if me % 2 == 0:
                            nc.scalar.activation(ot[:], ps[:], COPY)
                        else:
                            nc.vector.tensor_copy(ot[:], ps[:])
                        # sh=0 stores ride gpsimd mid-attention (cheap issue,
                        # exp stream untouched); sh=1 drain-tail stores stay
                        # off gpsimd (its DRAIN measured ~5us when it held
                        # the last stores).
                        dst = outT_d[P * me : P * (me + 1), q0 : q0 + w]
                        if sh == 0:
                            nc.gpsimd.dma_start(dst, ot[:])
                        else:
                            eng = (nc.sync, nc.scalar)[(me + hf) % 2]
                            eng.dma_start(dst, ot[:])

                me_sh0 = 0  # next out-proj column tile for the sh=0 half
                # prefill 5 deep (2 borrowed + 3 psp); steady state decays
                # back to 3-ahead so psp never holds more than 4 tiles
                for j in range(min(LOOKAHEAD + 2, len(work))):
                    issue_score(j)
                for j, (h, sh, t, i, ntiles) in enumerate(work):
                    if j >= 2 and j + LOOKAHEAD < len(work):
                        issue_score(j + LOOKAHEAD)
                    sc = scs.pop(j)
                    diag = (t, sh) in DIAG
                    c0 = P * t - 512 * sh if diag else 0
                    ex = exps.tile([P, 512], BF16, tag="ex")
                    bias = bias_t[:, t : t + 1] if (sh == 1 and t >= 4) else 0.0
                    nc.scalar.activation(ex[:, c0:512], sc[:, c0:512], EXP, bias=bias)
                    if diag:
                        # causal mask: zero the upper triangle of the
                        # diagonal 128x128 block (exact: 0 * finite = 0)
                        nc.vector.tensor_tensor(
                            ex[:, c0 : c0 + P], ex[:, c0 : c0 + P], tri_t[:], op=MULT
                        )
                    if i == 0:
                        ctx_ps = pscp.tile([P, 512], F32, tag="psc", name=f"ctxps_{h}_{sh}")
                        den_ps = psdp.tile([P, 512], F32, tag="psd", name=f"denps_{h}_{sh}")
                        groups[(h, sh)] = (ctx_ps, den_ps)
                    ctx_ps, den_ps = groups[(h, sh)]
                    first, last = (i == 0), (i == ntiles - 1)
                    nc.tensor.matmul(
                        ctx_ps[:, c0:512],
                        vsb[:, t, DH * h : DH * (h + 1)],
                        ex[:, c0:512],
                        start=first,
                        stop=last,
                    )
                    nc.tensor.matmul(
                        den_ps[:, c0:512], ones_t[:], ex[:, c0:512], start=first, stop=last
                    )
                    if last:
                        rc = rcp.tile([P, 512], F32, tag="rc")
                        nc.vector.reciprocal_approx_fast(rc[:], den_ps[:])
                        nc.vector.tensor_tensor(
                            ctxT[:, h, 512 * sh : 512 * (sh + 1)],
                            ctx_ps[:],
                            rc[:],
                            op=MULT,
                        )
                    # interleave the sh=0 output projection into the sh=1
                    # attention stream (2 column tiles per work item)
                    if sh == 1 and i >= ntiles - 2 and me_sh0 < 16:
                        out_proj(me_sh0, 0)
                        me_sh0 += 1

                while me_sh0 < 16:
                    out_proj(me_sh0, 0)
                    me_sh0 += 1
                for me in range(16):
                    out_proj(me, 1, nsplit=2 if me >= 14 else 1)
    nc.finalize()
    return nc


_NC_CACHE = {}


def get_nc(ktok=KTOK):
    if ktok not in _NC_CACHE:
        _NC_CACHE[ktok] = build_nc(ktok)
    return _NC_CACHE[ktok]


def make_in_maps(in_features, attention_mask, W_qkv, W_out):
    BF = ml_dtypes.bfloat16
    F8N = ml_dtypes.float8_e4m3
    x = np.asarray(in_features, np.float32)
    am = np.asarray(attention_mask)
    Wqkv = np.asarray(W_qkv, np.float32)
    Wout = np.asarray(W_out, np.float32)
    seq_lens = am.astype(np.int64).sum(-1)

    perm = np.concatenate([np.arange(0, DH, 2), np.arange(1, DH, 2)])
    Wqh = Wqkv[0:D].reshape(H, DH, D)
    Wkh = Wqkv[D : 2 * D].reshape(H, DH, D)
    Wvh = Wqkv[2 * D : 3 * D].reshape(H, DH, D)

    half = DH // 2
    freq = THETA ** (-2.0 * np.arange(half, dtype=np.float64) / DH)
    ang = np.arange(S, dtype=np.float64)[:, None] * freq  # [S, 64]
    cosv = np.cos(ang).T.astype(np.float32)  # [64, S]
    sinv = np.sin(ang).T.astype(np.float32)
    cs = np.empty([P, 2, S], np.float32)
    cs[0:64, 0] = cosv
    cs[64:128, 0] = cosv
    cs[0:64, 1] = sinv
    cs[64:128, 1] = sinv
    cs = cs.astype(BF)

    ones = np.ones([P, P], BF)
    pp = np.arange(P)[:, None]
    cc = np.arange(P)[None, :]
    tri = (pp <= cc).astype(BF)  # 1 on/above diagonal (sq >= sk allowed)

    in_maps = []
    for c in range(8):
        b, g = c // 2, c % 2
        hs = slice(g * HPC, (g + 1) * HPC)
        wq = Wqh[hs][:, perm, :].reshape(DC, D)
        wk = Wkh[hs][:, perm, :].reshape(DC, D)
        # wqk8[p, mt, j, i, m] = WqkT[256j+128i+p, 128mt+m] * WQK_SCALE
        wqkT = np.concatenate([wq, wk], 0).T * WQK_SCALE  # [D, 2048]
        wqk8 = np.ascontiguousarray(
            wqkT.reshape(8, 2, P, 16, P).transpose(2, 3, 0, 1, 4)
        ).astype(F8N)
        # wvm[p, mtv, kt, m] = WvT[128kt+p, 128mtv+m]
        wvT = Wvh[hs].reshape(DC, D).T  # [D, 1024]
        wvm = np.ascontiguousarray(
            wvT.reshape(16, P, 8, P).transpose(1, 2, 0, 3)
        ).astype(BF)
        xT = np.ascontiguousarray(x[b].T)  # [D, S] f32
        # x8[p, j, i, s] = xT[256j + 128i + p, s]
        x8 = np.ascontiguousarray(
            xT.reshape(8, 2, P, S).transpose(2, 0, 1, 3)
        ).astype(F8N)
        woT = Wout[:, g * DC : (g + 1) * DC].T.astype(BF)  # [DC, D]
        wom = np.ascontiguousarray(woT.reshape(8, P, D).transpose(1, 0, 2))

        sl = int(seq_lens[b])
        bias = np.zeros([P, 8], np.float32)
        for t in range(4, 8):
            bias[:, t] = np.where(t * P + np.arange(P) >= sl, NEG, 0.0)
        in_maps.append(
            dict(
                xT=xT.astype(BF),
                x8=x8,
                wqk8=wqk8,
                wvm=wvm,
                wom=wom,
                cs=cs,
                tri=tri,
                bias=bias,
                ones=ones,
            )
        )
    return in_maps


def kernel(in_features, past_k, past_v, attention_mask, W_qkv, W_out):
    seq_max = int(np.asarray(attention_mask).astype(np.int64).sum(-1).max())
    nc = get_nc(KTOK if seq_max <= KTOK else S)
    in_maps = make_in_maps(in_features, attention_mask, W_qkv, W_out)
    res = run_bass_kernel_spmd(nc, in_maps, core_ids=list(range(8)))
    out = np.empty((B, S, D), np.float32)
    for b in range(B):
        out[b] = (
            res.results[2 * b]["outT"].astype(np.float32)
            + res.results[2 * b + 1]["outT"].astype(np.float32)
        ).T
    return out


# revision 35
# speedup vs baseline: 1.0074x; 1.0074x over previous
"""Trainium2 Bass kernel for fused attention prefill (nn_Attn_50740743635107).

Reference computation (fp32):
  qkv = x @ W_qkv.T ; split q,k,v ; interleaved RoPE on q,k ;
  scores = q k^T / sqrt(dh) with causal+valid_k mask ; softmax ;
  ctx = attn @ v ; out = ctx @ W_out.T
Shapes: B=4, S=1024, D=2048, H=16, DH=128.

Sharding: 8 cores = 4 batches x 2 head-groups (8 heads each).
Each core computes a partial out^T [D, S] for its (batch, head-group);
the host sums the two head-group partials per batch and transposes.

v3 design notes (on top of the bf16 v2 pipeline):
- q/k projection runs in fp8e4m3 with MatmulPerfMode.DoubleRow: the
  contraction pairs two 128-d chunks per matmul ([128,2,N] moving APs),
  so the PE contracts 256 rows per 1-cycle column stream - 2x the bf16
  rate.  Measured on HW: a DoubleRow matmul costs the same ~1 cyc/col
  as bf16 while doing double the work; plain fp8 and 64-partition
  DoubleRow give no speedup, so scores stay bf16.
- fp8 error budget: e4m3 rounding is ~2.5% rms per operand.  q/k noise
  only perturbs softmax scores (common-mode cancels in the softmax
  ratio), measured end-to-end ~7e-3 rel err.  The v path (v-proj, ctx,
  out-proj) goes STRAIGHT to the output, where a single fp8 operand
  already measures ~2.5e-2 > 2e-2 tol, so it stays bf16.
- W_q/W_k are quantized at x256 (std 0.0156 -> 4.0; e4m3 subnormal
  floor is 2^-6) and descaled in the PSUM->SBUF copy scale; DH^-0.5
  rides the q copy scale.
- Everything else per v2: bf16 PE/DVE paths, f32 PSUM, host-side
  transposed layouts, k-projection truncated at 928 tokens, one exp
  per score tile with bias-mask, causal mask via tri-multiply on the
  diagonal block after exp, softmax denominator via ones-stationary
  matmul, reciprocal_approx_fast, PE p-state warm-up matmuls, sh-major
  attention with the sq<512 output projection interleaved in.

Scheduling (measured on HW, exec 324.5us -> 240.4us):
- The first ~20us is chip-aggregate DMA-bound (8 cores pulling inputs
  at once).  Only x8 + wqk8 issue early, spread over three queues; the
  v-proj inputs (xT bf16, wv) queue behind the late wqk8 tiles on
  gpsimd; wom rides sync alone.  40 warm-up matmuls bridge the landing
  window (also ramping the PE p-state).
- The warm-up SBUF tile's pool stays open through phase A: if it
  closed, the x8/wqk8 tiles would reuse its space and their DMAs would
  serialize behind the warm-up matmuls (cost: 14us, measured).
- qk-projection runs 2 m-tiles per block (psa bufs=6) so each landed
  x8 chunk feeds 4 matmuls.
- The last v-proj PSUM copies ride DVE so ACT is free for the first
  attention exps; drain-tail stores avoid gpsimd (slow queue DRAIN)
  and the last two output tiles split 512->2x256 cols to overlap
  copy/store with matmuls.
"""

import numpy as np
import ml_dtypes

import concourse.bass as bass
from concourse import bacc
import concourse.mybir as mybir
import concourse.tile as tile
from concourse.bass_utils import run_bass_kernel_spmd

B, S, D, H = 4, 1024, 2048, 16
DH = 128           # head dim
HPC = 8            # heads per core
DC = HPC * DH      # 1024: d-range per core
P = 128
KTOK = 928         # k tokens computed (>= max seq_len, multiple of 16)
THETA = 10000.0
NEG = -60.0        # additive mask for invalid-k positions
WQK_SCALE = 256.0  # fp8 quantization scale for W_q/W_k rows
F32 = mybir.dt.float32
BF16 = mybir.dt.bfloat16
F8 = mybir.dt.float8e4
DR = mybir.MatmulPerfMode.DoubleRow
MULT = mybir.AluOpType.mult
ADD = mybir.AluOpType.add
SUB = mybir.AluOpType.subtract
EXP = mybir.ActivationFunctionType.Exp
COPY = mybir.ActivationFunctionType.Copy

# score tiles per head: (sk_tile, sq_half) pairs that are (partially) allowed.
# sq_half h covers sq in [512h, 512h+512); sk tile t covers sk in [128t, ...).
# diagonal (need causal mask): sh=0: t=0..3 ; sh=1: t=4..7.
# full (no mask): sh=1: t=0..3.
DIAG = {(t, 0) for t in range(4)} | {(t, 1) for t in range(4, 8)}
ALLOWED = {0: [0, 1, 2, 3], 1: [0, 1, 2, 3, 4, 5, 6, 7]}  # sq_half -> sk tiles


def build_nc(ktok=KTOK):
    nc = bacc.Bacc()
    xT_d = nc.dram_tensor("xT", [D, S], BF16, kind="ExternalInput")
    # x8[p, j, i, s] = xT[256j + 128i + p, s] quantized e4m3: pair-interleaved
    # chunks for DoubleRow (contraction = 128 partitions x 2)
    x8_d = nc.dram_tensor("x8", [P, 8, 2, S], F8, kind="ExternalInput")
    # wqk8[p, mt, j, i, m] = WqkT[256j+128i+p, 128mt+m] * WQK_SCALE (e4m3)
    wqk8_d = nc.dram_tensor("wqk8", [P, 16, 8, 2, P], F8, kind="ExternalInput")
    # wvm[p, mtv, kt, m] = WvT[128kt+p, 128mtv+m] (bf16)
    wvm_d = nc.dram_tensor("wvm", [P, 8, 16, P], BF16, kind="ExternalInput")
    wom_d = nc.dram_tensor("wom", [P, 8, D], BF16, kind="ExternalInput")
    cs_d = nc.dram_tensor("cs", [P, 2, S], BF16, kind="ExternalInput")
    tri_d = nc.dram_tensor("tri", [P, P], BF16, kind="ExternalInput")
    bias_d = nc.dram_tensor("bias", [P, 8], F32, kind="ExternalInput")
    ones_d = nc.dram_tensor("ones", [P, P], BF16, kind="ExternalInput")
    outT_d = nc.dram_tensor("outT", [D, S], BF16, kind="ExternalOutput")

    with tile.TileContext(nc) as tc:
        with (
            tc.tile_pool(name="qkt", bufs=1) as qktp,      # [128,16,1024] bf16 32K/p
            tc.tile_pool(name="vsb", bufs=1) as vsbp,      # [128,8,1024] bf16 16K/p
            tc.tile_pool(name="cstb", bufs=1) as cstbp,    # tri/bias/ones consts
            # warm-up SBUF tile lives in the OUTER scope: if its pool closed,
            # phase A's x8/wqk8 tiles would reuse its space and their DMAs
            # would stall behind all the warm-up matmuls (measured 14us of
            # PE starvation from exactly that).
            tc.tile_pool(name="wsb", bufs=1) as wsbp,
        ):
            qkT = qktp.tile([P, 16, S], BF16, tag="qkt")
            vsb = vsbp.tile([P, 8, DC], BF16, tag="vsb")
            # consts on the gpsimd queue (cheap issue) so the scalar queue
            # starts streaming x8/wqk8 immediately
            tri_t = cstbp.tile([P, P], BF16, tag="tri")
            nc.gpsimd.dma_start(tri_t[:], tri_d[:])
            bias_t = cstbp.tile([P, 8], F32, tag="bias")
            nc.gpsimd.dma_start(bias_t[:], bias_d[:])
            ones_t = cstbp.tile([P, P], BF16, tag="ones")
            nc.gpsimd.dma_start(ones_t[:], ones_d[:])

            # ---- PE p-state warm-up: dummy matmuls on a memset tile (no DMA
            # dependency) while the input DMAs land. Output is never read.
            with (
                tc.tile_pool(name="warm", bufs=2, space=bass.MemorySpace.PSUM) as wps,
            ):
                warm_sb = wsbp.tile([P, 512], BF16, tag="wsb")
                nc.vector.memset(warm_sb[:], 0.5)
                for w in range(40):
                    wp = wps.tile([P, 512], F32, tag="warm")
                    nc.tensor.matmul(
                        wp[:], warm_sb[:, 0:P], warm_sb[:], start=True, stop=True
                    )

            # ================= phase A: QKV projection + RoPE =================
            with (
                tc.tile_pool(name="x8t", bufs=8) as x8p,     # 8x[128,2,1024] f8 2K/p
                tc.tile_pool(name="xt", bufs=16) as xtp,     # 16x[128,1024] bf16 32K/p
                tc.tile_pool(name="wqk", bufs=8) as wqkp,    # [128,8,2,128] f8 2K/p
                tc.tile_pool(name="wv", bufs=2) as wvp,      # [128,4,16,128] bf16 16K/p
                tc.tile_pool(name="cst", bufs=1) as cstp,    # cos/sin 4K/p
                tc.tile_pool(name="rope", bufs=2) as ropep,  # [128,1024] bf16 2K/p
                tc.tile_pool(name="psa", bufs=6, space=bass.MemorySpace.PSUM) as psap,
                tc.tile_pool(name="psv", bufs=2, space=bass.MemorySpace.PSUM) as psvp,
            ):
                # x8 as 8 SEPARATE tiles for per-chunk DMA dependencies: the
                # first DoubleRow matmul gates only on x8_0 + wqk8_0.
                x8s = [
                    x8p.tile([P, 2, S], F8, tag="x8t", name=f"x8_{j}")
                    for j in range(8)
                ]
                # bf16 x tiles feed the v projection (stationary slices)
                xts = [
                    xtp.tile([P, S], BF16, tag="xt", name=f"xt{kt}")
                    for kt in range(16)
                ]
                # zero the uncomputed k tail (tokens ktok..S of every k head)
                if ktok < S:
                    nc.vector.memset(qkT[:, 8:16, ktok:S], 0.0)
                # DMA plan: the first ~45us of HBM bandwidth is chip-aggregate
                # bound (8 cores pulling at once), and the qk-projection is
                # rate-limited by x8/wqk8 landing.  So ONLY x8 + wqk8 issue
                # early (scalar + gpsimd-after-consts); the v-proj inputs
                # (xts, wv) queue up on gpsimd BEHIND the late wqk8 tiles,
                # and wom rides sync alone (needed last).
                cs_t = cstp.tile([P, 2, S], BF16, tag="cs")
                nc.gpsimd.dma_start(cs_t[:], cs_d[:])
                wqks = []
                with tc.high_priority():
                    wqk_p = wqkp.tile([P, 8, 2, P], F8, tag="wqk", name="wqk_pre0")
                    nc.scalar.dma_start(wqk_p[:], wqk8_d[:, 0])
                    wqks.append(wqk_p)
                    # x8 spreads across three queues so parallel DGE streams
                    # land the qk-projection inputs faster
                    for j in range(8):
                        eng = (nc.scalar, nc.sync, nc.scalar, nc.gpsimd)[j % 4]
                        eng.dma_start(x8s[j][:], x8_d[:, j])
                    for pre in range(1, 6):
                        wqk_p = wqkp.tile(
                            [P, 8, 2, P], F8, tag="wqk", name=f"wqk_pre{pre}"
                        )
                        eng = nc.gpsimd if pre >= 3 else nc.scalar
                        eng.dma_start(wqk_p[:], wqk8_d[:, pre])
                        wqks.append(wqk_p)

                # ---- q/k projection in fp8 DoubleRow
                # (m-tile mt: 0..7 = q heads, 8..15 = k heads)
                # qkT[m, s] = sum_d wqkT[d, m] * xT[d, s]
                # Two m-tiles per block share each landed x8 chunk: 4 matmuls
                # (1.7us of PE work) per x8_j instead of 2, hiding the
                # early-window DMA landing rate.
                for blk in range(8):
                    mts = (2 * blk, 2 * blk + 1)
                    wqkts = []
                    for mt in mts:
                        if mt < 6:
                            wqkts.append(wqks[mt])
                        else:
                            w = wqkp.tile([P, 8, 2, P], F8, tag="wqk")
                            nc.gpsimd.dma_start(w[:], wqk8_d[:, mt])
                            wqkts.append(w)
                    is_k = mts[0] >= 8
                    n1 = (ktok - 512) if is_k else 512  # second-half cols
                    pss = [
                        (
                            psap.tile([P, 512], F32, tag="psa", name=f"qk{mt}_0"),
                            psap.tile([P, 512], F32, tag="psa", name=f"qk{mt}_1"),
                        )
                        for mt in mts
                    ]
                    for j in range(8):
                        for w, (ps0, ps1) in zip(wqkts, pss):
                            nc.tensor.matmul(
                                ps0[:], w[:, j], x8s[j][:, :, 0:512],
                                start=(j == 0), stop=(j == 7), perf_mode=DR,
                            )
                            nc.tensor.matmul(
                                ps1[:, 0:n1], w[:, j], x8s[j][:, :, 512 : 512 + n1],
                                start=(j == 0), stop=(j == 7), perf_mode=DR,
                            )
                    # descale fp8 W scale; q additionally gets 1/sqrt(dh)
                    csc = (1.0 / WQK_SCALE) * (1.0 if is_k else DH**-0.5)
                    for mt, (ps0, ps1) in zip(mts, pss):
                        nc.scalar.activation(
                            qkT[:, mt, 0:512], ps0[:], COPY, scale=csc
                        )
                        nc.scalar.activation(
                            qkT[:, mt, 512 : 512 + n1], ps1[:, 0:n1], COPY, scale=csc
                        )
                        # ---- RoPE in place on qkT[:, mt, :] once both halves
                        # done.  rows 0..63 = even dh (xe), 64..127 = odd (xo):
                        #   new_e = xe*cos - xo*sin ; new_o = xe*sin + xo*cos
                        tmp = ropep.tile([P, S], BF16, tag="rope")
                        col = qkT[:, mt, :]
                        nc.vector.tensor_tensor(
                            tmp[0:64, :], col[64:128, :], cs_t[64:128, 1, :], op=MULT
                        )
                        nc.vector.tensor_tensor(
                            tmp[64:128, :], col[0:64, :], cs_t[0:64, 1, :], op=MULT
                        )
                        nc.vector.tensor_tensor(col[:], col[:], cs_t[:, 0, :], op=MULT)
                        nc.vector.tensor_tensor(
                            col[0:64, :], col[0:64, :], tmp[0:64, :], op=SUB
                        )
                        nc.vector.tensor_tensor(
                            col[64:128, :], col[64:128, :], tmp[64:128, :], op=ADD
                        )

                # v-proj inputs issue AFTER the qk weights in the gpsimd
                # queue so they don't steal early HBM bandwidth; they land
                # by the time the PE finishes the 16 qk m-tiles.
                for kt in range(16):
                    nc.gpsimd.dma_start(xts[kt][:], xT_d[P * kt : P * (kt + 1), :])

                # ---- v projection: v[s, vd] = sum_d xT[d, s] * wvT[d, vd]
                # 512-wide moving chunks (full PSUM bank per matmul)
                for nh in range(2):
                    wv = wvp.tile([P, 4, 16, P], BF16, tag="wv")
                    nc.gpsimd.dma_start(wv[:], wvm_d[:, 4 * nh : 4 * (nh + 1)])
                    for st in range(8):
                        psv = psvp.tile([P, 512], F32, tag="psv")
                        for kt in range(16):
                            nc.tensor.matmul(
                                psv[:],
                                xts[kt][:, P * st : P * (st + 1)],
                                wv[:, :, kt, :],
                                start=(kt == 0),
                                stop=(kt == 15),
                            )
                        # the last few copies ride DVE so ACT is free for the
                        # first attention exps the moment their scores land
                        if nh == 1 and st >= 4:
                            nc.vector.tensor_copy(
                                vsb[:, st, 512 * nh : 512 * (nh + 1)], psv[:]
                            )
                        else:
                            nc.scalar.activation(
                                vsb[:, st, 512 * nh : 512 * (nh + 1)], psv[:], COPY
                            )

            # ============ phase B: attention + output projection ============
            with (
                tc.tile_pool(name="ctx", bufs=1) as ctxp,    # [128,8,1024] bf16 16K/p
                tc.tile_pool(name="ex", bufs=4) as exps,     # [128,512] bf16 1K/p
                tc.tile_pool(name="rc", bufs=2) as rcp,      # [128,512] f32 2K/p
                tc.tile_pool(name="wo", bufs=1) as wop,      # [128,8,2048] bf16 32K/p
                tc.tile_pool(name="ot", bufs=3) as otp,      # [128,512] bf16 1K/p
                tc.tile_pool(name="ps", bufs=4, space=bass.MemorySpace.PSUM) as psp,
                tc.tile_pool(name="psc", bufs=1, space=bass.MemorySpace.PSUM) as pscp,
                tc.tile_pool(name="psd", bufs=1, space=bass.MemorySpace.PSUM) as psdp,
                tc.tile_pool(name="pso", bufs=2, space=bass.MemorySpace.PSUM) as psop,
            ):
                ctxT = ctxp.tile([P, 8, S], BF16, tag="ctx")
                # W_out^T resident for the output projection
                wo_t = wop.tile([P, 8, D], BF16, tag="wo")
                nc.sync.dma_start(wo_t[:], wom_d[:])

                # ---- attention, software-pipelined and sh-major: scores
                # issue LOOKAHEAD items ahead of their exp/ctx/den; the sh=0
                # half of the output projection is interleaved into the sh=1
                # attention stream to keep the PE busy under the ACT chain.
                work = []  # (h, sh, t, i, ntiles)
                for sh in range(2):
                    for h in range(8):
                        tiles = ALLOWED[sh]
                        for i, t in enumerate(tiles):
                            work.append((h, sh, t, i, len(tiles)))

                LOOKAHEAD = 3
                scs = {}
                groups = {}  # (h, sh) -> (ctx_ps, den_ps)

                def issue_score(j):
                    h, sh, t, i, _n = work[j]
                    c0 = P * t - 512 * sh if (t, sh) in DIAG else 0
                    sc = psp.tile([P, 512], F32, tag="ps")
                    nc.tensor.matmul(
                        sc[:, c0:512],
                        qkT[:, 8 + h, P * t : P * (t + 1)],
                        qkT[:, h, 512 * sh + c0 : 512 * (sh + 1)],
                        start=True,
                        stop=True,
                    )
                    scs[j] = sc

                def out_proj(me, sh, nsplit=1):
                    # outT[e, sq] = sum_d woT[d, e] * ctxT[d, sq]
                    # nsplit=2 chops the 512 columns into two accumulation
                    # groups so the copy+store of the first half overlaps the
                    # second half's matmuls (shortens the drain tail).
                    w = 512 // nsplit
                    for hf in range(nsplit):
                        q0 = 512 * sh + w * hf
                        ps = psop.tile([P, w], F32, tag="pso")
                        for kd in range(8):
                            nc.tensor.matmul(
                                ps[:],
                                wo_t[:, kd, P * me : P * (me + 1)],
                                ctxT[:, kd, q0 : q0 + w],
                                start=(kd == 0),
                                stop=(kd == 7),
                            )
                        ot = otp.tile([P, w], BF16, tag="ot")
                        if me % 2 == 0:
                            nc.scalar.activation(ot[:], ps[:], COPY)
                        else:
                            nc.vector.tensor_copy(ot[:], ps[:])
                        # sh=0 stores ride gpsimd mid-attention (cheap issue,
                        # exp stream untouched); sh=1 drain-tail stores stay
                        # off gpsimd (its DRAIN measured ~5us when it held
                        # the last stores).
                        dst = outT_d[P * me : P * (me + 1), q0 : q0 + w]
                        if sh == 0:
                            nc.gpsimd.dma_start(dst, ot[:])
                        else:
                            eng = (nc.sync, nc.scalar)[(me + hf) % 2]
                            eng.dma_start(dst, ot[:])

                me_sh0 = 0  # next out-proj column tile for the sh=0 half
                for j in range(min(LOOKAHEAD, len(work))):
                    issue_score(j)
                for j, (h, sh, t, i, ntiles) in enumerate(work):
                    if j + LOOKAHEAD < len(work):
                        issue_score(j + LOOKAHEAD)
                    sc = scs.pop(j)
                    diag = (t, sh) in DIAG
                    c0 = P * t - 512 * sh if diag else 0
                    ex = exps.tile([P, 512], BF16, tag="ex")
                    bias = bias_t[:, t : t + 1] if (sh == 1 and t >= 4) else 0.0
                    nc.scalar.activation(ex[:, c0:512], sc[:, c0:512], EXP, bias=bias)
                    if diag:
                        # causal mask: zero the upper triangle of the
                        # diagonal 128x128 block (exact: 0 * finite = 0)
                        nc.vector.tensor_tensor(
                            ex[:, c0 : c0 + P], ex[:, c0 : c0 + P], tri_t[:], op=MULT
                        )
                    if i == 0:
                        ctx_ps = pscp.tile([P, 512], F32, tag="psc", name=f"ctxps_{h}_{sh}")
                        den_ps = psdp.tile([P, 512], F32, tag="psd", name=f"denps_{h}_{sh}")
                        groups[(h, sh)] = (ctx_ps, den_ps)
                    ctx_ps, den_ps = groups[(h, sh)]
                    first, last = (i == 0), (i == ntiles - 1)
                    nc.tensor.matmul(
                        ctx_ps[:, c0:512],
                        vsb[:, t, DH * h : DH * (h + 1)],
                        ex[:, c0:512],
                        start=first,
                        stop=last,
                    )
                    nc.tensor.matmul(
                        den_ps[:, c0:512], ones_t[:], ex[:, c0:512], start=first, stop=last
                    )
                    if last:
                        rc = rcp.tile([P, 512], F32, tag="rc")
                        nc.vector.reciprocal_approx_fast(rc[:], den_ps[:])
                        nc.vector.tensor_tensor(
                            ctxT[:, h, 512 * sh : 512 * (sh + 1)],
                            ctx_ps[:],
                            rc[:],
                            op=MULT,
                        )
                    # interleave the sh=0 output projection into the sh=1
                    # attention stream (2 column tiles per work item)
                    if sh == 1 and i >= ntiles - 2 and me_sh0 < 16:
                        out_proj(me_sh0, 0)
                        me_sh0 += 1

                while me_sh0 < 16:
                    out_proj(me_sh0, 0)
                    me_sh0 += 1
                for me in range(16):
                    out_proj(me, 1, nsplit=2 if me >= 14 else 1)
    nc.finalize()
    return nc


_NC_CACHE = {}


def get_nc(ktok=KTOK):
    if ktok not in _NC_CACHE:
        _NC_CACHE[ktok] = build_nc(ktok)
    return _NC_CACHE[ktok]


def make_in_maps(in_features, attention_mask, W_qkv, W_out):
    BF = ml_dtypes.bfloat16
    F8N = ml_dtypes.float8_e4m3
    x = np.asarray(in_features, np.float32)
    am = np.asarray(attention_mask)
    Wqkv = np.asarray(W_qkv, np.float32)
    Wout = np.asarray(W_out, np.float32)
    seq_lens = am.astype(np.int64).sum(-1)

    perm = np.concatenate([np.arange(0, DH, 2), np.arange(1, DH, 2)])
    Wqh = Wqkv[0:D].reshape(H, DH, D)
    Wkh = Wqkv[D : 2 * D].reshape(H, DH, D)
    Wvh = Wqkv[2 * D : 3 * D].reshape(H, DH, D)

    half = DH // 2
    freq = THETA ** (-2.0 * np.arange(half, dtype=np.float64) / DH)
    ang = np.arange(S, dtype=np.float64)[:, None] * freq  # [S, 64]
    cosv = np.cos(ang).T.astype(np.float32)  # [64, S]
    sinv = np.sin(ang).T.astype(np.float32)
    cs = np.empty([P, 2, S], np.float32)
    cs[0:64, 0] = cosv
    cs[64:128, 0] = cosv
    cs[0:64, 1] = sinv
    cs[64:128, 1] = sinv
    cs = cs.astype(BF)

    ones = np.ones([P, P], BF)
    pp = np.arange(P)[:, None]
    cc = np.arange(P)[None, :]
    tri = (pp <= cc).astype(BF)  # 1 on/above diagonal (sq >= sk allowed)

    in_maps = []
    for c in range(8):
        b, g = c // 2, c % 2
        hs = slice(g * HPC, (g + 1) * HPC)
        wq = Wqh[hs][:, perm, :].reshape(DC, D)
        wk = Wkh[hs][:, perm, :].reshape(DC, D)
        # wqk8[p, mt, j, i, m] = WqkT[256j+128i+p, 128mt+m] * WQK_SCALE
        wqkT = np.concatenate([wq, wk], 0).T * WQK_SCALE  # [D, 2048]
        wqk8 = np.ascontiguousarray(
            wqkT.reshape(8, 2, P, 16, P).transpose(2, 3, 0, 1, 4)
        ).astype(F8N)
        # wvm[p, mtv, kt, m] = WvT[128kt+p, 128mtv+m]
        wvT = Wvh[hs].reshape(DC, D).T  # [D, 1024]
        wvm = np.ascontiguousarray(
            wvT.reshape(16, P, 8, P).transpose(1, 2, 0, 3)
        ).astype(BF)
        xT = np.ascontiguousarray(x[b].T)  # [D, S] f32
        # x8[p, j, i, s] = xT[256j + 128i + p, s]
        x8 = np.ascontiguousarray(
            xT.reshape(8, 2, P, S).transpose(2, 0, 1, 3)
        ).astype(F8N)
        woT = Wout[:, g * DC : (g + 1) * DC].T.astype(BF)  # [DC, D]
        wom = np.ascontiguousarray(woT.reshape(8, P, D).transpose(1, 0, 2))

        sl = int(seq_lens[b])
        bias = np.zeros([P, 8], np.float32)
        for t in range(4, 8):
            bias[:, t] = np.where(t * P + np.arange(P) >= sl, NEG, 0.0)
        in_maps.append(
            dict(
                xT=xT.astype(BF),
                x8=x8,
                wqk8=wqk8,
                wvm=wvm,
                wom=wom,
                cs=cs,
                tri=tri,
                bias=bias,
                ones=ones,
            )
        )
    return in_maps


def kernel(in_features, past_k, past_v, attention_mask, W_qkv, W_out):
    seq_max = int(np.asarray(attention_mask).astype(np.int64).sum(-1).max())
    nc = get_nc(KTOK if seq_max <= KTOK else S)
    in_maps = make_in_maps(in_features, attention_mask, W_qkv, W_out)
    res = run_bass_kernel_spmd(nc, in_maps, core_ids=list(range(8)))
    out = np.empty((B, S, D), np.float32)
    for b in range(B):
        out[b] = (
            res.results[2 * b]["outT"].astype(np.float32)
            + res.results[2 * b + 1]["outT"].astype(np.float32)
        ).T
    return out
